# revision 1
# baseline (speedup 1.0000x reference)
"""MLA (multi-head latent attention) Bass kernel for Trainium2, 8 NeuronCores.

Sharding: core i handles batch b = i // 2 and head-group g = i % 2
(8 of the 16 heads).  Each core computes a partial output
(its heads' contribution through out_proj, plus b_o/2); the host sums
the two partials per batch.

Layout strategy (all on-chip tensors "t-major", i.e. feature dim on
partitions, sequence on the free axis):
  xT      [dim=8x128, S]   via PE (tensor-engine) transposes of x
  kv_latT [128, S]         = w_kvc^T @ xT        (+b_kvc)
  q_latT  [256, S]         = w_qc^T @ xT         (+b_qc)
  KT      [512, S]         = w_kvu_k^T @ kv_latT (+b)    (local heads)
  QT      [512, S]         = w_qu^T   @ q_latT   (+b)
  V       [S, 520]         = kv_lat @ w_kvu_v    (+b), 65-col blocks per
                             head: 64 value cols + a ones column.
Attention per (s-half j, head pair), streaming over key chunks k:
  scoresT[t,s] via matmul (head pair shares the PE array via disjoint
  64-row groups), exp(s/8) on ScalarE, causal handled by clipping the
  s-range + affine_select on the diagonal block; PV accumulates
  ctx^T[64, s] in PSUM, the ones column gives the softmax denominator
  in row 64.  ctx scaled by 1/denom (reciprocal + partition-broadcast
  multiply) into ctxT, then out = ctxT^T @ w_o + b_o/2.

Matmul operands use float32r (single-pass fp32 streaming on the PE,
4x faster than exact fp32); producers write tiles with f32r dtype so
operands are pre-rounded.
"""

import numpy as np

import concourse.bass as bass
import concourse.bacc as bacc
import concourse.mybir as mybir
import concourse.tile as tile
from concourse import masks

DIM = 1024
NUM_HEADS = 16
HEAD_DIM = 64
LAT = 128
QR = 256
B = 4
NCORES = 8
ND = DIM // 128       # 8 d-chunks
NHL = 8               # heads per core
F32 = mybir.dt.float32
F32R = mybir.dt.float32r
AF = mybir.ActivationFunctionType


def _pieces(total, w=512):
    return [(o, min(w, total - o)) for o in range(0, total, w)]


def build_mla(S=2048, mmdt=F32R):
    """Build the per-core Bass program (same SPMD program on all 8 cores)."""
    assert S % 256 == 0
    SH = S // 2           # s-half width
    NT = S // 128         # number of 128-token chunks

    nc = bacc.Bacc()

    x_d = nc.declare_dram_parameter("x", [S, DIM], F32, isOutput=False)
    w_kvc_d = nc.declare_dram_parameter("w_kvc", [DIM, LAT], F32, isOutput=False)
    w_qc_d = nc.declare_dram_parameter("w_qc", [DIM, QR], F32, isOutput=False)
    w_kvu_k_d = nc.declare_dram_parameter("w_kvu_k", [LAT, 512], F32, isOutput=False)
    w_kvu_v_d = nc.declare_dram_parameter("w_kvu_v", [LAT, 512], F32, isOutput=False)
    w_qu_d = nc.declare_dram_parameter("w_qu", [QR, 512], F32, isOutput=False)
    w_o_d = nc.declare_dram_parameter("w_o", [512, DIM], F32, isOutput=False)
    b_kvc_d = nc.declare_dram_parameter("b_kvc", [LAT, 1], F32, isOutput=False)
    b_qc_d = nc.declare_dram_parameter("b_qc", [128, 2], F32, isOutput=False)
    b_qu_d = nc.declare_dram_parameter("b_qu", [128, 4], F32, isOutput=False)
    b_kvu_k_d = nc.declare_dram_parameter("b_kvu_k", [128, 4], F32, isOutput=False)
    b_kvu_v_d = nc.declare_dram_parameter("b_kvu_v", [1, 512], F32, isOutput=False)
    b_o_d = nc.declare_dram_parameter("b_o", [1, DIM], F32, isOutput=False)
    out_d = nc.declare_dram_parameter("out", [S, DIM], F32, isOutput=True)

    with tile.TileContext(nc) as tc:
        with (
            tc.tile_pool(name="const", bufs=1) as const,
            tc.tile_pool(name="wts", bufs=1) as wts,
            tc.tile_pool(name="big", bufs=1) as big,
            tc.tile_pool(name="stg", bufs=2) as stg,
        ):
            ident = const.tile([128, 128], F32, name="ident")
            masks.make_identity(nc, ident[:])
            # memset doesn't support f32r; memset f32 then round-copy
            ones1f = const.tile([1, 128], F32, name="ones1f")
            nc.gpsimd.memset(ones1f[:], 1.0)
            ones1 = const.tile([1, 128], mmdt, name="ones1")
            nc.vector.tensor_copy(ones1[:], ones1f[:])

            # ---- weights into SBUF (staged fp32 DMA, rounded copy to mmdt) --
            def load_rounded(dst_ap, src_ap, shape):
                st = stg.tile([128, 1024], F32, tag="stage")
                sap = st[:shape[0], :shape[1]]
                nc.sync.dma_start(out=sap, in_=src_ap)
                nc.vector.tensor_copy(dst_ap, sap)

            w_kvc_sb = wts.tile([128, DIM], mmdt, name="w_kvc_sb")
            w_qc_sb = wts.tile([128, ND * QR], mmdt, name="w_qc_sb")
            for dc in range(ND):
                load_rounded(w_kvc_sb[:, 128 * dc:128 * dc + 128],
                             w_kvc_d[128 * dc:128 * dc + 128, :], (128, 128))
                load_rounded(w_qc_sb[:, QR * dc:QR * dc + QR],
                             w_qc_d[128 * dc:128 * dc + 128, :], (128, QR))
            w_kvu_k_sb = wts.tile([128, 512], mmdt, name="w_kvu_k_sb")
            load_rounded(w_kvu_k_sb[:], w_kvu_k_d[:, :], (128, 512))
            w_kvu_v_sb = wts.tile([128, 512], mmdt, name="w_kvu_v_sb")
            load_rounded(w_kvu_v_sb[:], w_kvu_v_d[:, :], (128, 512))
            w_qu_sb = wts.tile([128, 1024], mmdt, name="w_qu_sb")
            for qc in range(2):
                load_rounded(w_qu_sb[:, 512 * qc:512 * qc + 512],
                             w_qu_d[128 * qc:128 * qc + 128, :], (128, 512))
            b_kvu_v_sb = wts.tile([1, 512], mmdt, name="b_kvu_v_sb")
            load_rounded(b_kvu_v_sb[:], b_kvu_v_d[:, :], (1, 512))
            b_o_sb = wts.tile([1, DIM], mmdt, name="b_o_sb")
            load_rounded(b_o_sb[:], b_o_d[:, :], (1, DIM))
            # preload w_o so phase E starts without waiting on its DMA
            w_o_sb = wts.tile([128, 4 * DIM], mmdt, name="w_o_sb")
            for cc in range(4):
                load_rounded(w_o_sb[:, DIM * cc:DIM * cc + DIM],
                             w_o_d[128 * cc:128 * cc + 128, :], (128, DIM))

            # per-partition bias vectors (not matmul operands -> plain f32)
            b_kvc_sb = wts.tile([128, 1], F32, name="b_kvc_sb")
            nc.sync.dma_start(out=b_kvc_sb[:], in_=b_kvc_d[:, :])
            b_qc_sb = wts.tile([128, 2], F32, name="b_qc_sb")
            nc.sync.dma_start(out=b_qc_sb[:], in_=b_qc_d[:, :])
            b_qu_sb = wts.tile([128, 4], F32, name="b_qu_sb")
            nc.sync.dma_start(out=b_qu_sb[:], in_=b_qu_d[:, :])
            b_kvu_k_sb = wts.tile([128, 4], F32, name="b_kvu_k_sb")
            nc.sync.dma_start(out=b_kvu_k_sb[:], in_=b_kvu_k_d[:, :])

            # ---- persistent products: KT / QT / V (chunk c lives at cols c*S) ----
            KT = big.tile([128, 4 * S], mmdt, name="KT")
            QT = big.tile([128, 4 * S], mmdt, name="QT")
            V = big.tile([128, NT * 520], mmdt, name="V")
            # ones columns of V (col 64 of each 65-wide head block);
            # memset doesn't support f32r, so copy from an f32 ones tile
            v_view = V[:].rearrange("p (k h c) -> p k h c", h=NHL, c=65)
            ones_cols = const.tile([128, NT * NHL], F32, name="ones_cols")
            nc.gpsimd.memset(ones_cols[:], 1.0)
            nc.vector.tensor_copy(
                v_view[:, :, :, 64:65],
                ones_cols[:].rearrange("p (k h o) -> p k h o", h=NHL, o=1))

            # ================= phase A+B+C: transpose + projections =========
            with (
                tc.tile_pool(name="xin", bufs=3) as xin,
                tc.tile_pool(name="xtp", bufs=2) as xtp,
                tc.tile_pool(name="kvq", bufs=2) as kvq,
                tc.tile_pool(name="tpps", bufs=1, space="PSUM") as tpps,
                tc.tile_pool(name="pjps", bufs=1, space="PSUM") as pjps,
            ):
                for off, w in _pieces(S):
                    ntile = w // 128
                    # transpose x rows [off, off+w) -> xTp [128, 8 * w]
                    # (d-chunk dc at cols dc*w)
                    xTp = xtp.tile([128, ND * 512], mmdt, tag="xTp")
                    for q in range(ntile):
                        xt = xin.tile([128, DIM], F32, tag="xin")
                        nc.sync.dma_start(
                            out=xt[:],
                            in_=x_d[off + 128 * q:off + 128 * q + 128, :])
                        for dg in range(2):
                            ps = tpps.tile([128, 512], F32, tag="tp", bufs=2)
                            for u in range(4):
                                dc = 4 * dg + u
                                nc.tensor.transpose(
                                    ps[:, 128 * u:128 * u + 128],
                                    xt[:, 128 * dc:128 * dc + 128],
                                    ident[:])
                            dst = xTp[:].rearrange(
                                "p (d t) -> p d t", t=512
                            )[:, 4 * dg:4 * dg + 4, 128 * q:128 * q + 128]
                            src = ps[:].rearrange("p (d t) -> p d t", t=128)
                            nc.vector.tensor_copy(dst, src)
                    # kv_lat / q_lat for this piece
                    kvp = pjps.tile([128, 512], F32, tag="kv", bufs=1)
                    q0p = pjps.tile([128, 512], F32, tag="q0", bufs=1)
                    q1p = pjps.tile([128, 512], F32, tag="q1", bufs=1)
                    for dc in range(ND):
                        xr = xTp[:, dc * 512:dc * 512 + w]
                        st = dc == 0
                        sp = dc == ND - 1
                        nc.tensor.matmul(
                            kvp[:, :w], w_kvc_sb[:, 128 * dc:128 * dc + 128],
                            xr, start=st, stop=sp)
                        nc.tensor.matmul(
                            q0p[:, :w], w_qc_sb[:, QR * dc:QR * dc + 128],
                            xr, start=st, stop=sp)
                        nc.tensor.matmul(
                            q1p[:, :w], w_qc_sb[:, QR * dc + 128:QR * dc + 256],
                            xr, start=st, stop=sp)
                    kvs = kvq.tile([128, 512], mmdt, tag="kvs")
                    q0s = kvq.tile([128, 512], mmdt, tag="q0s")
                    q1s = kvq.tile([128, 512], mmdt, tag="q1s")
                    nc.vector.tensor_scalar_add(kvs[:, :w], kvp[:, :w], b_kvc_sb[:, 0:1])
                    nc.vector.tensor_scalar_add(q0s[:, :w], q0p[:, :w], b_qc_sb[:, 0:1])
                    nc.vector.tensor_scalar_add(q1s[:, :w], q1p[:, :w], b_qc_sb[:, 1:2])
                    # K^T / Q^T chunks for this piece
                    for c in range(4):
                        kp = pjps.tile([128, 512], F32, tag="pjo", bufs=2)
                        nc.tensor.matmul(
                            kp[:, :w], w_kvu_k_sb[:, 128 * c:128 * c + 128],
                            kvs[:, :w], start=True, stop=True)
                        nc.vector.tensor_scalar_add(
                            KT[:, c * S + off:c * S + off + w], kp[:, :w],
                            b_kvu_k_sb[:, c:c + 1])
                        qp = pjps.tile([128, 512], F32, tag="pjo", bufs=2)
                        nc.tensor.matmul(
                            qp[:, :w], w_qu_sb[:, 128 * c:128 * c + 128],
                            q0s[:, :w], start=True, stop=False)
                        nc.tensor.matmul(
                            qp[:, :w], w_qu_sb[:, 512 + 128 * c:512 + 128 * c + 128],
                            q1s[:, :w], start=False, stop=True)
                        nc.vector.tensor_scalar_add(
                            QT[:, c * S + off:c * S + off + w], qp[:, :w],
                            b_qu_sb[:, c:c + 1])
                    # V chunks for this piece
                    for q in range(ntile):
                        k = (off + 128 * q) // 128
                        vp = pjps.tile([128, 512], F32, tag="pjo", bufs=2)
                        nc.tensor.matmul(vp[:], ones1[0:1, :], b_kvu_v_sb[0:1, :],
                                         start=True, stop=False)
                        nc.tensor.matmul(vp[:], kvs[:, 128 * q:128 * q + 128],
                                         w_kvu_v_sb[:], start=False, stop=True)
                        nc.vector.tensor_copy(
                            v_view[:, k, :, 0:64],
                            vp[:].rearrange("p (h c) -> p h c", c=64))

            # ================= phase D: attention ===========================
            with tc.tile_pool(name="ctxTp", bufs=1) as ctxTp:
                ctxT = ctxTp.tile([128, 4 * S], mmdt, name="ctxT")
                with (
                    tc.tile_pool(name="attn", bufs=1) as attn,
                    tc.tile_pool(name="scps", bufs=1, space="PSUM") as scps,
                    tc.tile_pool(name="ctxps", bufs=2, space="PSUM") as ctxps,
                ):
                    nbank = (SH + 511) // 512
                    for j in range(2):
                        s0 = SH * j
                        kmax = (SH // 128) * (j + 1)
                        last_k = {
                            bi: min(kmax - 1, (s0 + 512 * (bi + 1)) // 128 - 1)
                            for bi in range(nbank)
                        }
                        for hp in range(NHL // 2):
                            heads = (2 * hp, 2 * hp + 1)
                            c = hp // 1  # KT/QT chunk = hp
                            ctxs = [ctxps.tile([65, SH], F32, tag="ctx",
                                               name=f"ctx{h}") for h in heads]
                            for k in range(kmax):
                                t0 = 128 * k
                                ss = max(s0, t0)
                                fd = s0 + SH - ss
                                rel = ss - s0
                                scs = []
                                # the two heads' QK matmuls are adjacent and
                                # use disjoint 64-row groups of the PE array
                                for o2, w2 in _pieces(fd):
                                    for hi, h in enumerate(heads):
                                        po = 64 * (h % 2)
                                        if o2 == 0:
                                            scs.append(scps.tile(
                                                [128, SH], F32, tag="sc",
                                                bufs=2, name=f"sc{h}"))
                                        nc.tensor.matmul(
                                            scs[hi][:, o2:o2 + w2],
                                            KT[po:po + 64,
                                               hp * S + t0:hp * S + t0 + 128],
                                            QT[po:po + 64,
                                               hp * S + ss + o2:hp * S + ss + o2 + w2],
                                            start=True, stop=True)
                                exs = []
                                for hi, h in enumerate(heads):
                                    ex = attn.tile([128, SH], mmdt, tag="ex",
                                                   bufs=4, name=f"ex{h}")
                                    exs.append(ex)
                                    nc.scalar.activation(ex[:, :fd], scs[hi][:, :fd],
                                                         AF.Exp, scale=0.125)
                                    if t0 >= s0:
                                        nc.gpsimd.affine_select(
                                            out=ex[:, 0:128], in_=ex[:, 0:128],
                                            pattern=[[1, 128]],
                                            compare_op=mybir.AluOpType.is_ge,
                                            fill=0.0, base=0, channel_multiplier=-1)
                                for hi, h in enumerate(heads):
                                    for bi in range(nbank):
                                        a = max(rel, 512 * bi)
                                        b2 = min(SH, 512 * bi + 512)
                                        if a >= b2:
                                            continue
                                        nc.tensor.matmul(
                                            ctxs[hi][:, a:b2],
                                            V[:, 520 * k + 65 * h:520 * k + 65 * h + 65],
                                            exs[hi][:, a - rel:b2 - rel],
                                            start=(k == 0), stop=(k == last_k[bi]))
                            # normalize: ctx[0:64] * (1/ctx[64])
                            for hi, h in enumerate(heads):
                                po = 64 * (h % 2)
                                rec = attn.tile([1, SH], F32, tag="rec", bufs=1,
                                                name=f"rec{h}")
                                nc.vector.reciprocal(rec[:], ctxs[hi][64:65, :])
                                rbc = attn.tile([64, SH], F32, tag="rbc", bufs=1,
                                                name=f"rbc{h}")
                                nc.gpsimd.partition_broadcast(rbc[:], rec[0:1, :])
                                nc.vector.tensor_mul(
                                    ctxT[po:po + 64, hp * S + s0:hp * S + s0 + SH],
                                    ctxs[hi][0:64, :], rbc[:])

            # ================= phase E: out projection ======================
                with (
                    tc.tile_pool(name="outsb", bufs=3) as outsb,
                    tc.tile_pool(name="ops", bufs=2, space="PSUM") as ops,
                ):
                    for si in range(NT):
                        op = ops.tile([128, DIM], F32, tag="op")
                        for o2, w2 in _pieces(DIM):
                            nc.tensor.matmul(op[:, o2:o2 + w2], ones1[0:1, :],
                                             b_o_sb[0:1, o2:o2 + w2],
                                             start=True, stop=False)
                        for cc in range(4):
                            for o2, w2 in _pieces(DIM):
                                nc.tensor.matmul(
                                    op[:, o2:o2 + w2],
                                    ctxT[:, cc * S + 128 * si:cc * S + 128 * si + 128],
                                    w_o_sb[:, DIM * cc + o2:DIM * cc + o2 + w2],
                                    start=False, stop=(cc == 3))
                        ob = outsb.tile([128, DIM], F32, tag="ob")
                        nc.vector.tensor_copy(ob[:, 0:512], op[:, 0:512])
                        nc.scalar.copy(ob[:, 512:DIM], op[:, 512:DIM])
                        nc.sync.dma_start(
                            out=out_d[128 * si:128 * si + 128, :], in_=ob[:])

    nc.finalize()
    return nc


def shard_inputs(inputs, S=2048):
    """Build the 8 per-core input maps from full inputs."""
    f = lambda a: np.ascontiguousarray(np.asarray(a, dtype=np.float32))
    x = f(inputs["x"])
    w_kvc, b_kvc = f(inputs["w_kvc"]), f(inputs["b_kvc"])
    w_kvu, b_kvu = f(inputs["w_kvu"]), f(inputs["b_kvu"])
    w_qc, b_qc = f(inputs["w_qc"]), f(inputs["b_qc"])
    w_qu, b_qu = f(inputs["w_qu"]), f(inputs["b_qu"])
    w_o, b_o = f(inputs["w_o"]), f(inputs["b_o"])
    in_maps = []
    for core in range(NCORES):
        b = core // 2
        g = core % 2
        cs = slice(512 * g, 512 * g + 512)
        in_maps.append({
            "x": x[b],
            "w_kvc": w_kvc,
            "w_qc": w_qc,
            "w_kvu_k": np.ascontiguousarray(w_kvu[:, 512 * g:512 * g + 512]),
            "w_kvu_v": np.ascontiguousarray(w_kvu[:, 1024 + 512 * g:1024 + 512 * g + 512]),
            "w_qu": np.ascontiguousarray(w_qu[:, cs]),
            "w_o": np.ascontiguousarray(w_o[cs, :]),
            "b_kvc": b_kvc.reshape(LAT, 1),
            "b_qc": np.ascontiguousarray(b_qc.reshape(2, 128).T),
            "b_qu": np.ascontiguousarray(b_qu[cs].reshape(4, 128).T),
            "b_kvu_k": np.ascontiguousarray(b_kvu[cs].reshape(4, 128).T),
            "b_kvu_v": np.ascontiguousarray(b_kvu[1024 + 512 * g:1024 + 512 * g + 512].reshape(1, 512)),
            "b_o": np.ascontiguousarray((b_o * 0.5).reshape(1, DIM)),
        })
    return in_maps


def kernel(**inputs) -> np.ndarray:
    from concourse.bass_utils import run_bass_kernel_spmd

    x = np.asarray(inputs["x"])
    S = x.shape[1]
    nc = build_mla(S=S)
    in_maps = shard_inputs(inputs, S=S)
    res = run_bass_kernel_spmd(nc, in_maps, list(range(NCORES))).results
    out = np.empty((B, S, DIM), dtype=np.float32)
    for b in range(B):
        out[b] = res[2 * b]["out"] + res[2 * b + 1]["out"]
    return out



# revision 4
# speedup vs baseline: 1.2019x; 1.2019x over previous
"""MLA (multi-head latent attention) Bass kernel for Trainium2, 8 NeuronCores.

Sharding: core i handles batch b = i // 2 and head-group g = i % 2
(8 of the 16 heads).  Each core computes a partial output (its heads'
contribution through out_proj); the host sums the two partials per batch
and adds a constant row (b_kvu_v @ w_o + b_o), which is exact because
softmax rows sum to 1 so the V-bias passes through attention additively.

All matmul operands are bf16 (1 cycle/row on the PE regardless of
output width); accumulation stays f32 in PSUM.  No PE transposes: both
x -> xT and ctx -> ctxT go through the DMA XBAR (dma_start_transpose,
2-byte dtypes) after an f32->bf16 rounding copy on GpSimd/DVE.

Pipeline (single TileContext; emission interleaved so attention starts
~20us in and out_proj overlaps the second attention half):
  piece(p), p=0..3 (512 tokens each):
    x chunks DMA'd, rounded to bf16 on Pool, DMA-transposed into
    xT [128, 8 d-chunks, S]; latents kv_latT [128,S], q_latT{0,1}
    [128,S] = W^T xT (+bias, DVE); KT/QT [128, 4 chunks * S] and
    V [128, NT*520] (64 cols/head + ones col for the softmax denom).
  attention(j, hp) per s-half j and head pair hp, heads sequential:
    scoresT [128 keys, 1024 queries] per key-chunk k via PE (64-row
    operands, disjoint groups per head); exp on ScalarE (scale=1/8,
    bf16 out); causal diagonal via affine_select on Pool; PV re-uses
    exp tiles as stationary: ctx_psum[s-chunk] [128 queries, 65]
    accumulates over k with the ones column giving the denominator.
    Retire: strided reciprocal [128,8] + 8 per-partition scalar
    multiplies (DVE) into a token-major bf16 pair tile, then one DMA
    transpose per (j,hp) into ctxT [128, 4 chunks * S].
  out_proj per 128-token chunk: 4x128-contraction accumulate into
  [128,512] PSUM halves, copies split DVE/Pool, DMA out.
"""

import numpy as np

import concourse.bass as bass
import concourse.bacc as bacc
import concourse.mybir as mybir
import concourse.tile as tile

DIM = 1024
NUM_HEADS = 16
HEAD_DIM = 64
LAT = 128
QR = 256
B = 4
NCORES = 8
ND = DIM // 128       # 8 d-chunks
NHL = 8               # heads per core
F32 = mybir.dt.float32
BF16 = mybir.dt.bfloat16
AF = mybir.ActivationFunctionType


def _pieces(total, w=512):
    return [(o, min(w, total - o)) for o in range(0, total, w)]


def build_mla(S=2048, mmdt=BF16):
    """Build the per-core Bass program (same SPMD program on all 8 cores)."""
    assert S % 512 == 0
    SH = S // 2           # s-half width
    NT = S // 128         # number of 128-token chunks
    NP = S // 512         # number of 512-token pieces

    nc = bacc.Bacc()

    x_d = nc.declare_dram_parameter("x", [S, DIM], F32, isOutput=False)
    w_kvc_d = nc.declare_dram_parameter("w_kvc", [DIM, LAT], F32, isOutput=False)
    w_qc_d = nc.declare_dram_parameter("w_qc", [DIM, QR], F32, isOutput=False)
    w_kvu_k_d = nc.declare_dram_parameter("w_kvu_k", [LAT, 512], F32, isOutput=False)
    w_kvu_v_d = nc.declare_dram_parameter("w_kvu_v", [LAT, 512], F32, isOutput=False)
    w_qu_d = nc.declare_dram_parameter("w_qu", [QR, 512], F32, isOutput=False)
    w_o_d = nc.declare_dram_parameter("w_o", [512, DIM], F32, isOutput=False)
    b_kvc_d = nc.declare_dram_parameter("b_kvc", [LAT, 1], F32, isOutput=False)
    b_qc_d = nc.declare_dram_parameter("b_qc", [128, 2], F32, isOutput=False)
    b_qu_d = nc.declare_dram_parameter("b_qu", [128, 4], F32, isOutput=False)
    b_kvu_k_d = nc.declare_dram_parameter("b_kvu_k", [128, 4], F32, isOutput=False)
    out_d = nc.declare_dram_parameter("out", [S, DIM], F32, isOutput=True)

    with tile.TileContext(nc) as tc:
        with (
            tc.tile_pool(name="wts", bufs=1) as wts,
            tc.tile_pool(name="big", bufs=1) as big,
            tc.tile_pool(name="stg", bufs=2) as stg,
            tc.tile_pool(name="xfp", bufs=2) as xfp,
            tc.tile_pool(name="xbp", bufs=2) as xbp,
            tc.tile_pool(name="attn", bufs=1) as attn,
            tc.tile_pool(name="cpp", bufs=2) as cpp,
            tc.tile_pool(name="obp", bufs=3) as obp,
            tc.tile_pool(name="scps", bufs=1, space="PSUM") as scps,
            tc.tile_pool(name="ctxps", bufs=1, space="PSUM") as ctxps,
        ):
            # ---- persistent products -----------------------------------
            xT = big.tile([128, ND * S], mmdt, name="xT")
            xT_v = xT[:].rearrange("p (d t) -> p d t", d=ND)
            kv_latT = big.tile([128, S], mmdt, name="kv_latT")
            q_latT0 = big.tile([128, S], mmdt, name="q_latT0")
            q_latT1 = big.tile([128, S], mmdt, name="q_latT1")
            KT = big.tile([128, 4 * S], mmdt, name="KT")
            QT = big.tile([128, 4 * S], mmdt, name="QT")
            V = big.tile([128, NT * 520], mmdt, name="V")
            v_view = V[:].rearrange("p (k h c) -> p k h c", h=NHL, c=65)
            ctxT = big.tile([128, 4 * S], mmdt, name="ctxT")
            ctxT_v = ctxT[:].rearrange("p (c t) -> p c t", c=4)

            # ones columns of V (col 64 of each 65-wide head block)
            nc.gpsimd.memset(v_view[:, :, :, 64:65], 1.0)

            # ---- weights into SBUF (staged fp32 DMA, rounded to bf16) --
            def load_rounded(dst_ap, src_ap, shape):
                st = stg.tile([128, 1024], F32, tag="stage")
                sap = st[:shape[0], :shape[1]]
                nc.sync.dma_start(out=sap, in_=src_ap)
                nc.vector.tensor_copy(dst_ap, sap)

            w_kvc_sb = wts.tile([128, DIM], mmdt, name="w_kvc_sb")
            w_qc_sb = wts.tile([128, ND * QR], mmdt, name="w_qc_sb")
            for dc in range(ND):
                load_rounded(w_kvc_sb[:, 128 * dc:128 * dc + 128],
                             w_kvc_d[128 * dc:128 * dc + 128, :], (128, 128))
                load_rounded(w_qc_sb[:, QR * dc:QR * dc + QR],
                             w_qc_d[128 * dc:128 * dc + 128, :], (128, QR))
            w_kvu_k_sb = wts.tile([128, 512], mmdt, name="w_kvu_k_sb")
            load_rounded(w_kvu_k_sb[:], w_kvu_k_d[:, :], (128, 512))
            w_kvu_v_sb = wts.tile([128, 512], mmdt, name="w_kvu_v_sb")
            load_rounded(w_kvu_v_sb[:], w_kvu_v_d[:, :], (128, 512))
            w_qu_sb = wts.tile([128, 1024], mmdt, name="w_qu_sb")
            for qc in range(2):
                load_rounded(w_qu_sb[:, 512 * qc:512 * qc + 512],
                             w_qu_d[128 * qc:128 * qc + 128, :], (128, 512))
            w_o_sb = wts.tile([128, 4 * DIM], mmdt, name="w_o_sb")
            for cc in range(4):
                load_rounded(w_o_sb[:, DIM * cc:DIM * cc + DIM],
                             w_o_d[128 * cc:128 * cc + 128, :], (128, DIM))

            # per-partition bias vectors (DVE scalar operands, f32)
            b_kvc_sb = wts.tile([128, 1], F32, name="b_kvc_sb")
            nc.sync.dma_start(out=b_kvc_sb[:], in_=b_kvc_d[:, :])
            b_qc_sb = wts.tile([128, 2], F32, name="b_qc_sb")
            nc.sync.dma_start(out=b_qc_sb[:], in_=b_qc_d[:, :])
            b_qu_sb = wts.tile([128, 4], F32, name="b_qu_sb")
            nc.sync.dma_start(out=b_qu_sb[:], in_=b_qu_d[:, :])
            b_kvu_k_sb = wts.tile([128, 4], F32, name="b_kvu_k_sb")
            nc.sync.dma_start(out=b_kvu_k_sb[:], in_=b_kvu_k_d[:, :])

            # ---- emission helpers --------------------------------------
            def piece(pj, p):
                """x transpose + all projections for tokens [512p, 512p+512)."""
                o = 512 * p
                for q in range(4 * p, 4 * p + 4):
                    xf = xfp.tile([128, DIM], F32, tag="xf")
                    nc.sync.dma_start(
                        out=xf[:], in_=x_d[128 * q:128 * q + 128, :])
                    xb = xbp.tile([128, DIM], mmdt, tag="xb")
                    nc.gpsimd.tensor_copy(xb[:], xf[:])
                    nc.sync.dma_start_transpose(
                        xT_v[:, :, 128 * q:128 * q + 128], xb[:])
                # latents
                kvp = pj.tile([128, 512], F32, tag="pj")
                for dc in range(ND):
                    nc.tensor.matmul(
                        kvp[:], w_kvc_sb[:, 128 * dc:128 * dc + 128],
                        xT_v[:, dc, o:o + 512], start=(dc == 0), stop=(dc == ND - 1))
                nc.vector.tensor_scalar_add(
                    kv_latT[:, o:o + 512], kvp[:], b_kvc_sb[:, 0:1])
                for half, qlat in ((0, q_latT0), (1, q_latT1)):
                    qp = pj.tile([128, 512], F32, tag="pj")
                    for dc in range(ND):
                        nc.tensor.matmul(
                            qp[:], w_qc_sb[:, QR * dc + 128 * half:QR * dc + 128 * half + 128],
                            xT_v[:, dc, o:o + 512], start=(dc == 0), stop=(dc == ND - 1))
                    nc.vector.tensor_scalar_add(
                        qlat[:, o:o + 512], qp[:], b_qc_sb[:, half:half + 1])
                # K^T / Q^T chunks
                for c in range(4):
                    kp = pj.tile([128, 512], F32, tag="pj")
                    nc.tensor.matmul(
                        kp[:], w_kvu_k_sb[:, 128 * c:128 * c + 128],
                        kv_latT[:, o:o + 512], start=True, stop=True)
                    nc.vector.tensor_scalar_add(
                        KT[:, c * S + o:c * S + o + 512], kp[:],
                        b_kvu_k_sb[:, c:c + 1])
                    qp2 = pj.tile([128, 512], F32, tag="pj")
                    nc.tensor.matmul(
                        qp2[:], w_qu_sb[:, 128 * c:128 * c + 128],
                        q_latT0[:, o:o + 512], start=True, stop=False)
                    nc.tensor.matmul(
                        qp2[:], w_qu_sb[:, 512 + 128 * c:512 + 128 * c + 128],
                        q_latT1[:, o:o + 512], start=False, stop=True)
                    nc.vector.tensor_scalar_add(
                        QT[:, c * S + o:c * S + o + 512], qp2[:],
                        b_qu_sb[:, c:c + 1])
                # V chunks
                for q in range(4 * p, 4 * p + 4):
                    vp = pj.tile([128, 512], F32, tag="pj")
                    nc.tensor.matmul(vp[:], kv_latT[:, 128 * q:128 * q + 128],
                                     w_kvu_v_sb[:], start=True, stop=True)
                    nc.vector.tensor_copy(
                        v_view[:, q, :, 0:64],
                        vp[:].rearrange("p (h c) -> p h c", c=64))

            def attn_unit(j, hp):
                """Attention for s-half j, head pair hp (heads sequential)."""
                s0 = SH * j
                kmax = (s0 + SH) // 128
                cp = cpp.tile([128, 1024], mmdt, tag="cp")
                for h2 in range(2):
                    h = 2 * hp + h2
                    po = 64 * h2
                    ctx = ctxps.tile([128, 1024], F32, tag="ctx", bufs=1)
                    exs = []
                    for k in range(kmax):
                        t0 = 128 * k
                        ss = max(s0, t0)
                        fd = s0 + SH - ss
                        sc = scps.tile([128, SH], F32, tag="sc", bufs=2)
                        for o2, w2 in _pieces(fd):
                            nc.tensor.matmul(
                                sc[:, o2:o2 + w2],
                                KT[po:po + 64, hp * S + t0:hp * S + t0 + 128],
                                QT[po:po + 64, hp * S + ss + o2:hp * S + ss + o2 + w2],
                                start=True, stop=True)
                        ex = attn.tile([128, SH], mmdt, tag="ex", bufs=18)
                        exs.append(ex)
                        nc.scalar.activation(ex[:, :fd], sc[:, :fd],
                                             AF.Exp, scale=0.125)
                        if t0 >= s0:
                            nc.gpsimd.affine_select(
                                out=ex[:, 0:128], in_=ex[:, 0:128],
                                pattern=[[1, 128]],
                                compare_op=mybir.AluOpType.is_ge,
                                fill=0.0, base=0, channel_multiplier=-1)
                    # PV: one contiguous accumulation group per s-chunk
                    # (PSUM banks support only one open group at a time)
                    for c in range(8):
                        klast = 8 * j + c
                        for k in range(klast + 1):
                            rel = max(s0, 128 * k) - s0
                            cs = 128 * c - rel
                            nc.tensor.matmul(
                                ctx[:, 128 * c:128 * c + 65],
                                exs[k][:, cs:cs + 128],
                                V[:, 520 * k + 65 * h:520 * k + 65 * h + 65],
                                start=(k == 0), stop=(k == klast))
                    # retire head: rec = 1/denom, scale 64 ctx cols per chunk
                    rec = attn.tile([128, 8], F32, tag="rec", bufs=2)
                    nc.vector.reciprocal(
                        rec[:],
                        ctx[:].rearrange("p (c u) -> p c u", u=128)[:, :, 64])
                    for c in range(8):
                        nc.vector.tensor_scalar_mul(
                            cp[:, 128 * c + po:128 * c + po + 64],
                            ctx[:, 128 * c:128 * c + 64], rec[:, c:c + 1])
                nc.sync.dma_start_transpose(
                    ctxT_v[:, hp, s0:s0 + SH].rearrange("p (b t) -> p b t", t=128),
                    cp[:])

            def out_chunk(ops, si):
                """out_proj for tokens [128si, 128si+128)."""
                ob = obp.tile([128, DIM], F32, tag="ob")
                for u in range(2):
                    op = ops.tile([128, 512], F32, tag="op", bufs=2)
                    for cc in range(4):
                        nc.tensor.matmul(
                            op[:],
                            ctxT_v[:, cc, 128 * si:128 * si + 128],
                            w_o_sb[:, DIM * cc + 512 * u:DIM * cc + 512 * u + 512],
                            start=(cc == 0), stop=(cc == 3))
                    nc.vector.tensor_copy(ob[:, 512 * u:512 * u + 512], op[:])
                nc.sync.dma_start(
                    out=out_d[128 * si:128 * si + 128, :], in_=ob[:])

            # ---- emission schedule -------------------------------------
            with tc.tile_pool(name="pjps", bufs=1, space="PSUM") as pj:
                piece(pj, 0)
                piece(pj, 1)
                attn_unit(0, 0)
                piece(pj, 2)
                attn_unit(0, 1)
                piece(pj, 3)
                attn_unit(0, 2)
                attn_unit(0, 3)
            with tc.tile_pool(name="ops", bufs=1, space="PSUM") as ops:
                for hp in range(4):
                    attn_unit(1, hp)
                    out_chunk(ops, 2 * hp)
                    out_chunk(ops, 2 * hp + 1)
                for si in range(8, NT):
                    out_chunk(ops, si)

    nc.finalize()
    return nc


def shard_inputs(inputs, S=2048):
    """Build the 8 per-core input maps from full inputs."""
    f = lambda a: np.ascontiguousarray(np.asarray(a, dtype=np.float32))
    x = f(inputs["x"])
    w_kvc, b_kvc = f(inputs["w_kvc"]), f(inputs["b_kvc"])
    w_kvu, b_kvu = f(inputs["w_kvu"]), f(inputs["b_kvu"])
    w_qc, b_qc = f(inputs["w_qc"]), f(inputs["b_qc"])
    w_qu, b_qu = f(inputs["w_qu"]), f(inputs["b_qu"])
    w_o = f(inputs["w_o"])
    in_maps = []
    for core in range(NCORES):
        b = core // 2
        g = core % 2
        cs = slice(512 * g, 512 * g + 512)
        in_maps.append({
            "x": x[b],
            "w_kvc": w_kvc,
            "w_qc": w_qc,
            "w_kvu_k": np.ascontiguousarray(w_kvu[:, 512 * g:512 * g + 512]),
            "w_kvu_v": np.ascontiguousarray(w_kvu[:, 1024 + 512 * g:1024 + 512 * g + 512]),
            "w_qu": np.ascontiguousarray(w_qu[:, cs]),
            "w_o": np.ascontiguousarray(w_o[cs, :]),
            "b_kvc": b_kvc.reshape(LAT, 1),
            "b_qc": np.ascontiguousarray(b_qc.reshape(2, 128).T),
            "b_qu": np.ascontiguousarray(b_qu[cs].reshape(4, 128).T),
            "b_kvu_k": np.ascontiguousarray(b_kvu[cs].reshape(4, 128).T),
        })
    return in_maps


def gather_out(results, inputs, S=2048):
    """Sum the two per-batch partials and add the constant bias row."""
    f = lambda a: np.asarray(a, dtype=np.float32)
    b_v = f(inputs["b_kvu"])[DIM:]
    const_row = b_v @ f(inputs["w_o"]) + f(inputs["b_o"])
    out = np.empty((B, S, DIM), dtype=np.float32)
    for b in range(B):
        out[b] = results[2 * b]["out"] + results[2 * b + 1]["out"] + const_row
    return out


def kernel(**inputs) -> np.ndarray:
    from concourse.bass_utils import run_bass_kernel_spmd

    x = np.asarray(inputs["x"])
    S = x.shape[1]
    nc = build_mla(S=S)
    in_maps = shard_inputs(inputs, S=S)
    res = run_bass_kernel_spmd(nc, in_maps, list(range(NCORES))).results
    return gather_out(res, inputs, S=S)
